# revision 18
# baseline (speedup 1.0000x reference)
"""Trainium2 Bass kernel for a belief-transformer block.

Computation (per batch b):
    h   = LayerNorm(x[b]) * g1
    qkv = h @ w_qkv ; q,k,v = split(qkv)
    s   = q @ k^T / sqrt(D), keys j >= L_b masked
    y   = softmax(s) @ v
    y   = LayerNorm(y) * g2
    out = gelu(y @ w_fc) @ w_proj

Sharding: data-parallel over batch across 8 NeuronCores (4 batches/core),
weights replicated.

Device-side structure per batch (tokens n, features d; P=128 partitions):
  A: LN1 stats+apply token-major          [n_chunk(128), d(512)]
  B: PE-transpose h -> h_T feature-major, interleaved with the v matmuls
     (key mask folded into the v copy-back)
  C: q_T,k_T feature-major
  then per token-half (512 queries) to bound SBUF:
  D: s_T = k @ q^T  (keys on partitions), exp on ACT -> p_T
  E: y_unnorm = p_T^T @ v_masked (token-major); rowsums r via a
     mask-column matmul into a [2, 512] PSUM row + PE-transpose shuffle
  F: LN2 on y_unnorm in-place; softmax normalization is absorbed by LN
     invariance, with the exact eps correction rsqrt(ssq/D + eps*r^2)
  G: PE-transpose y_ln -> y_ln_T, interleaved into neighboring matmul
     streams (PE transposes don't count as busy for the HAM clock gate)
  H: z_T = gelu(w_fc^T @ y_ln_T) feature-major
  I: out = z_T^T @ w_proj token-major -> DMA out

Matmuls run in float32r (full-rate, ~1e-4 rel err), accumulation fp32 in PSUM.
"""

import os
import sys

import numpy as np


def _ensure_concourse():
    try:
        import concourse  # noqa: F401
        return
    except ImportError:
        pass
    for p in ("/root/.axon_site/_ro/trn_rl_repo", "/opt/trn_rl_repo"):
        if os.path.isdir(p) and p not in sys.path:
            sys.path.insert(0, p)
    import concourse  # noqa: F401


_ensure_concourse()

import concourse.tile as tile  # noqa: E402
from concourse import bacc, mybir  # noqa: E402
from concourse.bass_utils import run_bass_kernel_spmd  # noqa: E402
from concourse.masks import make_identity  # noqa: E402

B, N, D = 32, 1024, 512
NCORES = 8
G = B // NCORES  # batches per core
P = 128
NT = N // P      # token chunks (8)
DC = D // P      # feature chunks (4)
HT_ = N // 2     # token half (512)
HC = HT_ // P    # token chunks per half (4)
EPS = 1e-5

F32 = mybir.dt.float32
F32R = mybir.dt.float32r
ALU = mybir.AluOpType
ACT = mybir.ActivationFunctionType


def _body(ctx, tc, x, msk, wdram, out, warm):
    nc = tc.nc

    singles = ctx.enter_context(tc.tile_pool(name="singles", bufs=1))
    main = ctx.enter_context(tc.tile_pool(name="main", bufs=1))
    xpool = ctx.enter_context(tc.tile_pool(name="xpool", bufs=2))
    outp = ctx.enter_context(tc.tile_pool(name="outp", bufs=2))
    stats = ctx.enter_context(tc.tile_pool(name="stats", bufs=2))
    ps_mm = ctx.enter_context(tc.tile_pool(name="ps_mm", bufs=4, space="PSUM"))
    ps_t = ctx.enter_context(tc.tile_pool(name="ps_t", bufs=3, space="PSUM"))
    ps_r = ctx.enter_context(tc.tile_pool(name="ps_r", bufs=1, space="PSUM"))

    # Replicated weights, feature-chunked [P, DC, D]; float32r for the PE.
    # wv is loaded first (feeds the PE warm-up and the earliest matmuls);
    # batch 0's x/mask DMAs are emitted before the remaining weights so the
    # LN1 critical path is not queued behind 5 MB of weight traffic.
    W = {}

    def load_w(name):
        t = singles.tile([P, DC, D], F32R, tag=name, name=name)
        nc.sync.dma_start(
            t[:], wdram[name].rearrange("(c p) e -> p c e", p=P).bitcast(F32R)
        )
        W[name] = t

    ident = singles.tile([P, P], F32, tag="ident")
    make_identity(nc, ident)
    eps_t = singles.tile([P, 1], F32, tag="eps")
    nc.vector.memset(eps_t[:], EPS)

    S = [dict() for _ in range(G)]  # per-batch live tiles

    def emit_A(b):
        """Load x/mask, LN1 -> H (DVE/ACT only)."""
        s = S[b]
        xb = x[b].rearrange("(t p) d -> p t d", p=P)
        mb = msk[b].rearrange("(t p) -> p t", p=P)
        X = xpool.tile([P, NT, D], F32, tag="X")
        nc.sync.dma_start(X[:], xb)
        s["mask_f"] = xpool.tile([P, NT], F32, tag="mask_f", name="mask_f")
        nc.sync.dma_start(s["mask_f"][:], mb)
        mask_s = xpool.tile([P, NT], F32R, tag="mask_s")
        nc.sync.dma_start(mask_s[:], mb.bitcast(F32R))
        s["mask_r"] = xpool.tile([P, NT, 2], F32R, tag="mask_r", name="mask_r")
        nc.vector.tensor_copy(
            s["mask_r"][:], mask_s[:, :, None].to_broadcast((P, NT, 2))
        )
        s["H"] = main.tile([P, NT, D], F32, tag="H", name="H")
        for t in range(NT):
            st = stats.tile([P, 6], F32, tag="bnst")
            nc.vector.bn_stats(st[:], X[:, t, :])
            mv = stats.tile([P, 2], F32, tag="bnag")
            nc.vector.bn_aggr(mv[:], st[:])
            sd = stats.tile([P, 1], F32, tag="sd")
            nc.scalar.activation(sd[:], mv[:, 1:2], ACT.Sqrt, bias=eps_t[:])
            rstd = stats.tile([P, 1], F32, tag="rstd")
            nc.vector.reciprocal(rstd[:], sd[:])
            nc.vector.tensor_scalar(
                s["H"][:, t, :], X[:, t, :], mv[:, 0:1], rstd[:],
                op0=ALU.subtract, op1=ALU.mult,
            )

    def emit_B_alloc(b):
        s = S[b]
        s["HT"] = main.tile([P, DC, N], F32R, tag="HT", name="HT")
        s["VM"] = main.tile([P, NT, D], F32R, tag="VM", name="VM")

    def emit_Bt(b, t):
        """Transpose h chunk t -> h_T, and the v-matmul for chunk t
        (interleaves real matmuls into the transpose burst for HAM)."""
        s = S[b]
        for c in range(DC):
            pt = ps_t.tile([P, P], F32, tag="pst")
            nc.tensor.transpose(pt[:], s["H"][:, t, c * P:(c + 1) * P], ident[:])
            nc.vector.tensor_copy(s["HT"][:, c, t * P:(t + 1) * P], pt[:])
        pm = ps_mm.tile([P, 512], F32, tag="psmm")
        for dc_ in range(DC):
            nc.tensor.matmul(
                pm[:],
                s["HT"][:, dc_, t * P:(t + 1) * P],
                W["wv"][:, dc_, :],
                start=(dc_ == 0), stop=(dc_ == DC - 1),
            )
        # mask keys >= L_b by zeroing their v rows during the copy-back
        nc.vector.tensor_scalar_mul(s["VM"][:, t, :], pm[:], s["mask_f"][:, t:t + 1])

    def emit_C_alloc(b):
        s = S[b]
        s["QT"] = main.tile([P, DC, N], F32R, tag="QT", name="QT")
        s["KT"] = main.tile([P, DC, N], F32R, tag="KT", name="KT")

    def emit_C_piece(b, h, c, which):
        """One PSUM group of the q_T/k_T production (4 matmuls)."""
        s = S[b]
        wt, tt = ((W["wq"], s["QT"]), (W["wk"], s["KT"]))[which]
        pm = ps_mm.tile([P, 512], F32, tag="psmm")
        for dc_ in range(DC):
            nc.tensor.matmul(
                pm[:],
                wt[:, dc_, c * P:(c + 1) * P],
                s["HT"][:, dc_, h * 512:(h + 1) * 512],
                start=(dc_ == 0), stop=(dc_ == DC - 1),
            )
        nc.scalar.copy(tt[:, c, h * 512:(h + 1) * 512], pm[:])

    def emit_C_half(b, h):
        for which in range(2):
            for c in range(DC):
                emit_C_piece(b, h, c, which)

    def emit_D_alloc(b, hf):
        S[b][f"PT{hf}"] = main.tile([P, NT, HT_], F32R, tag="PT", name="PT")

    def emit_D(b, hf, jc):
        """Scores for key-chunk jc (keys on partitions) + exp."""
        s = S[b]
        q0 = hf * HT_
        pm = ps_mm.tile([P, 512], F32, tag="psmm")
        for dc_ in range(DC):
            nc.tensor.matmul(
                pm[:],
                s["KT"][:, dc_, jc * P:(jc + 1) * P],
                s["QT"][:, dc_, q0:q0 + HT_],
                start=(dc_ == 0), stop=(dc_ == DC - 1),
            )
        nc.scalar.activation(s[f"PT{hf}"][:, jc, :], pm[:], ACT.Exp)

    def emit_E(b, hf):
        """y_unnorm = p^T @ v_masked; rowsums r via mask-column matmuls."""
        s = S[b]
        PT = s[f"PT{hf}"]
        Y = main.tile([P, HC, D], F32, tag="Y", name="Y")
        R = stats.tile([P, HC], F32, tag="R", name="R")
        s[f"Y{hf}"], s[f"R{hf}"] = Y, R
        pr2 = ps_r.tile([2, HT_], F32, tag="psr2", name="psr2")
        for jc in range(NT):
            nc.tensor.matmul(
                pr2[:],
                s["mask_r"][:, jc, :],
                PT[:, jc, :],
                start=(jc == 0), stop=(jc == NT - 1),
            )
        rrow = stats.tile([2, HT_], F32, tag="rrow", name="rrow")
        nc.vector.tensor_copy(rrow[:], pr2[:])
        # cross-partition shuffle [1, 512] -> [128, HC] via PE transposes
        for c in range(HC):
            ptr = ps_r.tile([P, 2], F32, tag="psr2", name="ptr")
            nc.tensor.transpose(
                ptr[:], rrow[:, c * P:(c + 1) * P], ident[0:2, 0:2]
            )
            nc.vector.tensor_copy(R[:, c:c + 1], ptr[:, 0:1])
        for il in range(HC):
            pm = ps_mm.tile([P, 512], F32, tag="psmm")
            for jc in range(NT):
                nc.tensor.matmul(
                    pm[:],
                    PT[:, jc, il * P:(il + 1) * P],
                    s["VM"][:, jc, :],
                    start=(jc == 0), stop=(jc == NT - 1),
                )
            nc.vector.tensor_copy(Y[:, il, :], pm[:])

    def emit_F(b, hf):
        """LN2 in-place on Y (absorbs softmax norm; exact eps via r^2).
        Stats via DVE bn_stats; ACT only does Sqrt (no table thrash)."""
        s = S[b]
        Y, R = s[f"Y{hf}"], s[f"R{hf}"]
        mvs, sds = [], []
        for il in range(HC):
            st2 = stats.tile([P, 6], F32, tag="bnst")
            nc.vector.bn_stats(st2[:], Y[:, il, :])
            mv2 = stats.tile([P, 2], F32, tag="bnag2")
            nc.vector.bn_aggr(mv2[:], st2[:])
            mvs.append(mv2)
        for il in range(HC):
            epsr2 = stats.tile([P, 1], F32, tag="epsr2")
            nc.vector.tensor_tensor(
                epsr2[:], R[:, il:il + 1], R[:, il:il + 1], ALU.mult
            )
            nc.vector.tensor_scalar(
                epsr2[:], epsr2[:], EPS, None, op0=ALU.mult
            )
            sd2 = stats.tile([P, 1], F32, tag="sd2")
            nc.scalar.activation(
                sd2[:], mvs[il][:, 1:2], ACT.Sqrt, bias=epsr2[:]
            )
            sds.append(sd2)
        for il in range(HC):
            rstd2 = stats.tile([P, 1], F32, tag="rstd2")
            nc.vector.reciprocal(rstd2[:], sds[il][:])
            nc.vector.tensor_scalar(
                Y[:, il, :], Y[:, il, :], mvs[il][:, 0:1], rstd2[:],
                op0=ALU.subtract, op1=ALU.mult,
            )

    def emit_G_alloc(b, hf):
        S[b][f"YLT{hf}"] = main.tile([P, DC, HT_], F32R, tag="YLT", name="YLT")

    def emit_G_pair(b, hf, k):
        """Two of the 16 y_ln transposes (k in 0..7)."""
        s = S[b]
        Y, YLT = s[f"Y{hf}"], s[f"YLT{hf}"]
        for idx in (2 * k, 2 * k + 1):
            tl, c = divmod(idx, DC)
            pt = ps_t.tile([P, P], F32, tag="pst")
            nc.tensor.transpose(pt[:], Y[:, tl, c * P:(c + 1) * P], ident[:])
            nc.vector.tensor_copy(YLT[:, c, tl * P:(tl + 1) * P], pt[:])

    def emit_HI(b, hf):
        """fc + gelu feature-major, then proj token-major + store."""
        s = S[b]
        YLT = s[f"YLT{hf}"]
        ob = out[b].rearrange("(t p) d -> p t d", p=P)
        ZT = main.tile([P, DC, HT_], F32R, tag="ZT", name="ZT")
        for c in range(DC):
            pm = ps_mm.tile([P, 512], F32, tag="psmm")
            for ec in range(DC):
                nc.tensor.matmul(
                    pm[:],
                    W["wf"][:, ec, c * P:(c + 1) * P],
                    YLT[:, ec, :],
                    start=(ec == 0), stop=(ec == DC - 1),
                )
            nc.scalar.activation(ZT[:, c, :], pm[:], ACT.Gelu)
        for il in range(HC):
            pm = ps_mm.tile([P, 512], F32, tag="psmm")
            for c in range(DC):
                nc.tensor.matmul(
                    pm[:],
                    ZT[:, c, il * P:(il + 1) * P],
                    W["wp"][:, c, :],
                    start=(c == 0), stop=(c == DC - 1),
                )
            o = outp.tile([P, D], F32, tag="O")
            nc.vector.tensor_copy(o[:], pm[:])
            nc.sync.dma_start(ob[:, hf * HC + il, :], o[:])

    # --- batch pipeline with transposes interleaved into matmul streams ---
    a_done = [False] * G
    bc_done = [False] * G
    for b in range(G):
        if b == 0:
            # startup: x/mask first, then wv + PE warm-up, then the rest
            load_w("wv")
            emit_A(0)

            def warm_burst(k0, n_mm, last):
                wpm = ps_mm.tile([P, 512], F32, tag="psmm", name="warmmm")
                for k in range(n_mm):
                    nc.tensor.matmul(
                        wpm[:], W["wv"][:, (k0 + k) % DC, 0:P],
                        W["wv"][:, (k0 + k) % DC, :],
                        start=(k == 0), stop=(k == n_mm - 1),
                    )
                if last:
                    wsb = outp.tile([P, 8], F32, tag="O", name="warmsb")
                    nc.vector.tensor_copy(wsb[:], wpm[:, 0:8])
                    nc.sync.dma_start(warm[:], wsb[:])

            warm_burst(0, 10, False)
            for name in ("wq", "wk", "wf", "wp"):
                load_w(name)
            a_done[0] = True
            emit_B_alloc(0)
            emit_C_alloc(0)
            for t in range(NT):
                emit_Bt(0, t)
                if t == 0:
                    warm_burst(10, 8, False)
                elif t == 1:
                    warm_burst(18, 8, True)
                if t >= HC:
                    emit_C_piece(0, 0, t - HC, 0)
                    emit_C_piece(0, 0, t - HC, 1)
            emit_C_half(0, 1)
            bc_done[0] = True
        # half 0: scores
        emit_D_alloc(b, 0)
        for jc in range(NT):
            emit_D(b, 0, jc)
        emit_E(b, 0)
        emit_F(b, 0)
        # half 1 scores interleaved with half-0 y_ln transposes
        emit_D_alloc(b, 1)
        emit_G_alloc(b, 0)
        if b + 1 < G:
            emit_A(b + 1)  # DVE work, overlaps the D(h1)/E(h1) PE stream
            a_done[b + 1] = True
        for jc in range(NT):
            emit_D(b, 1, jc)
            emit_G_pair(b, 0, jc)
        emit_HI(b, 0)
        emit_E(b, 1)
        emit_F(b, 1)
        emit_G_alloc(b, 1)
        if b + 1 < G:
            # tail: next batch's transposes + v/qk matmuls + this G(h1)
            emit_B_alloc(b + 1)
            emit_C_alloc(b + 1)
            for t in range(NT):
                emit_Bt(b + 1, t)
                if t >= HC:
                    emit_C_piece(b + 1, 0, t - HC, 0)
                    emit_C_piece(b + 1, 0, t - HC, 1)
            for k2 in range(NT):
                emit_G_pair(b, 1, k2)
                emit_C_piece(b + 1, 1, k2 % DC, k2 // DC)
            bc_done[b + 1] = True
        else:
            for k2 in range(NT):
                emit_G_pair(b, 1, k2)
        emit_HI(b, 1)
